# revision 5
# baseline (speedup 1.0000x reference)
"""Trainium2 8-core GCN kernel (2-layer GCNConv + linear head + softmax).

Strategy (node/row partitioning, dense normalized adjacency):
  - Host: build Ahat = D^-1/2 (A+I) D^-1/2 as a dense fp8-e4m3 matrix, padded
    from 10000 to 10240 nodes; core k owns node rows [k*1280, (k+1)*1280).
  - Device, per core k (bf16 GEMMs, fp8 DoubleRow SpMMs, fp32 accumulate):
      t1     = x @ W1 for ALL nodes (replicated GEMM; cheaper than the
               all-gather + reload stall it replaces)
      h1T_k  = relu(t1^T Ahat^T[:,k] + b1)    (transposed SpMM -> [512,1280])
      t2_k   = (h1T_k)^T @ W2                 (h1T is directly the lhsT)
      t2     = AllGather(t2_k) in 3 chunks (512/512/256 rows), each launched
               as soon as the SpMM1 column pass feeding it is evacuated;
               SpMM2 consumes j-pairs in chunk-arrival order
      h2T_k  = relu(t2^T Ahat^T[:,k] + b2)
      out_k  = softmax(h2T_k^T @ Wout + bout) ([1280, 16] f32)
  - Host: concatenate core outputs, trim padding to [10000, 16].

The transposed SpMM (z^T = t^T A^T instead of z = A t) makes each layer's
activation land in [feature, node] layout, which is exactly the lhsT the
following GEMM needs -- no on-device transposes anywhere.  The SpMM runs in
fp8 e4m3 with perf_mode=DoubleRow (contraction 256 rows per matmul):
lhsT/rhs are [128, 2, free] j-pair tiles, element [p, q] = row q*128+p.
"""

import contextlib
import ctypes
import sys
import types

import ml_dtypes
import numpy as np

import concourse.bass as bass
import concourse.mybir as mybir
import concourse.tile as tile
from concourse.bass_utils import run_bass_kernel_spmd

BF16 = ml_dtypes.bfloat16
FP8 = ml_dtypes.float8_e4m3

N_CORES = 8
N_NODES = 10000
F_IN = 512
F_HID = 512
N_CLASSES = 16
NP = 10240            # padded node count (80 * 128)
R = NP // N_CORES     # 1280 rows per core
P = 128
NJ = NP // P          # 80 contraction chunks
NJP = NJ // 2         # 40 DoubleRow contraction pairs
NM = R // P           # 10 row tiles per core
NF = F_HID // P       # 4 feature tiles
# SpMM node-column passes (aligned with the GEMM2/all-gather chunks)
MC_PASSES = [(0, 512), (512, 512), (1024, 256)]
AG_ROWS = [512, 512, 256]            # all-gather chunk sizes (rows of ag_in)
AG_OFF = [0, 512, 1024]
AG_MTILES = [range(0, 4), range(4, 8), range(8, 10)]

_NTFF_HOOK_INSTALLED = False


def install_ntff_hook():
    """bass_utils' trace=True path wants antenv.axon_hooks; this container
    doesn't ship it, so provide the same ctypes hook trn_boot would."""
    global _NTFF_HOOK_INSTALLED
    if _NTFF_HOOK_INSTALLED:
        return
    _NTFF_HOOK_INSTALLED = True
    try:
        lib = ctypes.CDLL("/opt/axon/libaxon_pjrt.so")
        if not hasattr(lib, "axon_start_nrt_profile"):
            return
    except OSError:
        return
    lib.axon_start_nrt_profile.argtypes = [
        ctypes.POINTER(ctypes.c_int64),
        ctypes.c_size_t,
    ]
    lib.axon_start_nrt_profile.restype = ctypes.c_int64
    lib.axon_stop_nrt_profile.argtypes = [ctypes.c_char_p]
    lib.axon_stop_nrt_profile.restype = ctypes.c_int64

    @contextlib.contextmanager
    def _hook(output_dir, device_ids):
        import jax

        jax.devices()
        if device_ids:
            ids = (ctypes.c_int64 * len(device_ids))(*device_ids)
            rc = lib.axon_start_nrt_profile(ids, len(device_ids))
        else:
            rc = lib.axon_start_nrt_profile(None, 0)
        if rc != 0:
            raise RuntimeError(f"axon_start_nrt_profile rc={rc}")
        try:
            yield
        finally:
            n = lib.axon_stop_nrt_profile(str(output_dir).encode())
            print(f"ntff profile: {n} file(s) -> {output_dir}", file=sys.stderr)

    import antenv

    mod = types.ModuleType("antenv.axon_hooks")
    mod.get_axon_ntff_profile_hook = lambda: _hook
    mod.set_axon_ntff_profile_hook = lambda h: None
    sys.modules["antenv.axon_hooks"] = mod
    antenv.axon_hooks = mod


def split_drain_waits(nc):
    """This walrus build allows only ONE sync-wait per lowered instruction
    (CTRL and pseudo-DMA structs assert on more).  Tile's wait-assignment can
    attach several; keep the last wait on the instruction and move the rest
    onto preceding single-wait NoOps on the same engine stream (waits are
    monotonic >= conditions, so enforcing them earlier in program order on
    the same engine is equivalent)."""
    for f in nc.m.functions:
        for bb in f.blocks:
            insts = bb.instructions
            i = 0
            while i < len(insts):
                inst = insts[i]
                si = getattr(inst, "sync_info", None)
                if si is not None and si.on_wait and len(si.on_wait) > 1:
                    waits = list(si.on_wait)
                    si.on_wait = [waits[-1]]
                    for j, w in enumerate(waits[:-1]):
                        pre = mybir.InstNoOp(
                            name=f"{inst.name}-presync-{j}",
                            engine=inst.engine,
                            ins=[],
                            outs=[],
                            sync_info=mybir.SyncInfo(on_wait=[w], on_update=[]),
                        )
                        insts.insert(i + j, pre)
                        nc.register_instruction(pre, overwrite=True)
                    i += len(waits) - 1
                i += 1


def build_gcn(nc):
    """Emit the SPMD GCN program (identical on every core; per-core data)."""
    f32 = mybir.dt.float32
    bf16 = mybir.dt.bfloat16
    fp8 = mybir.dt.float8e4
    rg = [list(range(N_CORES))]

    # I/O (per-core shards; same names on every core)
    # xTt[j, p, c*128+m] = x[j*128+m, c*128+p] -- pre-tiled full x^T blocks
    xTt = nc.declare_dram_parameter("xTt", [NJ, P, F_IN], bf16, isOutput=False)
    # ATdr[jp, p, q, m] = AhatT[jp*256 + q*128 + p, k*R + m]  (fp8 pairs)
    ATdr = nc.declare_dram_parameter("ATdr", [NJP, P, 2, R], fp8, isOutput=False)
    W1 = nc.declare_dram_parameter("W1", [F_IN, F_HID], bf16, isOutput=False)
    W2 = nc.declare_dram_parameter("W2", [F_HID, F_HID], bf16, isOutput=False)
    Wout = nc.declare_dram_parameter("Wout", [F_HID, N_CLASSES], bf16, isOutput=False)
    bcols = nc.declare_dram_parameter("bcols", [P, 2 * NF], f32, isOutput=False)
    bout = nc.declare_dram_parameter("bout", [1, N_CLASSES], bf16, isOutput=False)
    out = nc.declare_dram_parameter("out", [R, N_CLASSES], f32, isOutput=True)

    # layer-2 collective bounce buffers (internal DRAM), 3 chunks, fp8
    ag_in = nc.dram_tensor("ag_in", [R, F_HID], fp8)
    ag_out = [
        nc.dram_tensor(
            f"ag_out{c}", [N_CORES * AG_ROWS[c], F_HID], fp8, addr_space="Shared"
        )
        for c in range(3)
    ]

    with tile.TileContext(nc) as tc:
        with (
            tc.tile_pool(name="const", bufs=1) as cpool,
            tc.tile_pool(name="tfull", bufs=1) as tpool,
            tc.tile_pool(name="hT", bufs=1) as hpool,
            tc.tile_pool(name="work", bufs=4) as wpool,
            tc.tile_pool(name="evac", bufs=4) as epool,
            tc.tile_pool(name="sm", bufs=4) as spool,
            tc.tile_pool(name="psum", bufs=1, space="PSUM") as ppool,
        ):
            # ---- resident constants ----
            W1_sb = [cpool.tile([P, F_HID], bf16, tag=f"W1{c}", name=f"W1{c}") for c in range(NF)]
            W2_sb = [cpool.tile([P, F_HID], bf16, tag=f"W2{c}", name=f"W2{c}") for c in range(NF)]
            for c in range(NF):
                nc.sync.dma_start(out=W1_sb[c][:], in_=W1[c * P:(c + 1) * P, :])
                nc.sync.dma_start(out=W2_sb[c][:], in_=W2[c * P:(c + 1) * P, :])
            Wout_sb = [cpool.tile([P, N_CLASSES], bf16, tag=f"Wo{c}", name=f"Wo{c}") for c in range(NF)]
            for c in range(NF):
                nc.sync.dma_start(out=Wout_sb[c][:], in_=Wout[c * P:(c + 1) * P, :])
            bcols_sb = cpool.tile([P, 2 * NF], f32, tag="bcols", name="bcols")
            nc.sync.dma_start(out=bcols_sb[:], in_=bcols[:, :])
            bout_sb = cpool.tile([1, N_CLASSES], bf16, tag="bout", name="bout")
            nc.sync.dma_start(out=bout_sb[:], in_=bout[:, :])
            ones_sb = cpool.tile([1, P], bf16, tag="ones", name="ones")
            nc.vector.memset(ones_sb[:], 1.0)

            # persistent activation tiles: j-PAIR tiles for DoubleRow
            t_pair = [
                tpool.tile([P, 2, F_HID], fp8, tag=f"tp{jp}", name=f"tp{jp}")
                for jp in range(NJP)
            ]
            hT = [
                [hpool.tile([P, R], bf16, tag=f"h{la}T{c}", name=f"h{la}T{c}") for c in range(NF)]
                for la in range(2)
            ]

            # ---- layer 1: replicated GEMM1 (bf16), t1 for ALL nodes ----
            for j in range(NJ):
                xt = wpool.tile([P, F_IN], bf16, tag="xtt", name="xtt")
                nc.sync.dma_start(out=xt[:], in_=xTt[j, :, :])
                ps = ppool.tile([P, F_HID], f32, tag=f"sp{j % 4}", name=f"g1ps{j % 4}")
                for c in range(NF):
                    nc.tensor.matmul(
                        out=ps[:],
                        lhsT=xt[:, c * P:(c + 1) * P],
                        rhs=W1_sb[c][:],
                        start=(c == 0),
                        stop=(c == NF - 1),
                    )
                nc.vector.tensor_copy(out=t_pair[j // 2][:, j % 2, :], in_=ps[:])

            def spmm_pass(layer, mc, jp_order, names):
                """One fp8 DoubleRow column pass of hT[layer] = relu(tᵀAᵀ+b)."""
                off, width = mc
                pstiles = [
                    ppool.tile([P, width], f32, tag=f"sp{f}", name=f"{names}_{f}")
                    for f in range(NF)
                ]
                for idx, jp in enumerate(jp_order):
                    at = wpool.tile([P, 2, width], fp8, tag=f"at{off}", name=f"at{off}")
                    nc.scalar.dma_start(
                        out=at[:], in_=ATdr[jp, :, :, off:off + width]
                    )
                    for f in range(NF):
                        nc.tensor.matmul(
                            out=pstiles[f][:],
                            lhsT=t_pair[jp][:, :, f * P:(f + 1) * P],
                            rhs=at[:, :, :],
                            start=(idx == 0),
                            stop=(idx == NJP - 1),
                            perf_mode=mybir.MatmulPerfMode.DoubleRow,
                        )
                # evacuate: relu(psum + b) -> bf16, b is a per-partition col
                for f in range(NF):
                    nc.vector.tensor_scalar(
                        out=hT[layer][f][:, off:off + width],
                        in0=pstiles[f][:],
                        scalar1=bcols_sb[:, layer * NF + f:layer * NF + f + 1],
                        scalar2=0.0,
                        op0=mybir.AluOpType.add,
                        op1=mybir.AluOpType.max,
                    )

            def gemm2_tiles(ms):
                """t2_k rows for m-tiles `ms` staged into ag_in (as fp8)."""
                for m in ms:
                    ps = ppool.tile([P, F_HID], f32, tag=f"sp{4 + m % 4}", name=f"g2ps{m % 4}")
                    for c in range(NF):
                        nc.tensor.matmul(
                            out=ps[:],
                            lhsT=hT[0][c][:, m * P:(m + 1) * P],
                            rhs=W2_sb[c][:],
                            start=(c == 0),
                            stop=(c == NF - 1),
                        )
                    ev = epool.tile([P, F_HID], fp8, tag="g2ev", name="g2ev")
                    nc.vector.tensor_copy(out=ev[:], in_=ps[:])
                    nc.sync.dma_start(out=ag_in[m * P:(m + 1) * P, :], in_=ev[:])

            def ag_chunk(c):
                nc.gpsimd.collective_compute(
                    "AllGather",
                    mybir.AluOpType.bypass,
                    replica_groups=rg,
                    ins=[ag_in[AG_OFF[c]:AG_OFF[c] + AG_ROWS[c], :].opt()],
                    outs=[ag_out[c][:, :].opt()],
                )

            def load_t2_chunk(c):
                # ag_out[c] rows r*AG_ROWS[c]+i*128 -> t_pair j = r*10 + base+i
                ntiles = AG_ROWS[c] // P
                for r in range(N_CORES):
                    for i in range(ntiles):
                        j = r * NM + AG_OFF[c] // P + i
                        row = r * AG_ROWS[c] + i * P
                        nc.sync.dma_start(
                            out=t_pair[j // 2][:, j % 2, :],
                            in_=ag_out[c][row:row + P, :],
                        )

            # SpMM2 consumes j-pairs in chunk-arrival order:
            # chunk c gives, per rank r, j tiles r*10 + AG_OFF[c]/128 + i
            # -> pairs jp = (r*10 + AG_OFF[c]//128)/2 + i/2  (all aligned)
            order2 = []
            for c in range(3):
                for r in range(N_CORES):
                    base = r * NM + AG_OFF[c] // P
                    for jp in range(base // 2, (base + AG_ROWS[c] // P) // 2):
                        order2.append(jp)
            assert sorted(order2) == list(range(NJP))

            natural = list(range(NJP))

            # ---- layer 1 SpMM passes, GEMM2 + AG chunks interleaved ----
            for c, mc in enumerate(MC_PASSES):
                spmm_pass(0, mc, natural, f"s1p{c}")
                gemm2_tiles(AG_MTILES[c])
                ag_chunk(c)
            for c in range(3):
                load_t2_chunk(c)

            # ---- layer 2 SpMM ----
            for c, mc in enumerate(MC_PASSES):
                spmm_pass(1, mc, order2, f"s2p{c}")

            # ---- output head: logits + softmax ----
            for m in range(NM):
                ps = ppool.tile([P, N_CLASSES], f32, tag=f"sp{4 + m % 4}", name=f"hps{m % 4}")
                for c in range(NF):
                    nc.tensor.matmul(
                        out=ps[:],
                        lhsT=hT[1][c][:, m * P:(m + 1) * P],
                        rhs=Wout_sb[c][:],
                        start=(c == 0),
                        stop=False,
                    )
                nc.tensor.matmul(
                    out=ps[:],
                    lhsT=ones_sb[:, 0:P],
                    rhs=bout_sb[:],
                    start=False,
                    stop=True,
                )
                negmax = spool.tile([P, 1], f32, tag="negmax", name="negmax")
                nc.vector.tensor_reduce(
                    out=negmax[:], in_=ps[:], axis=mybir.AxisListType.X,
                    op=mybir.AluOpType.max, negate=True,
                )
                ex = spool.tile([P, N_CLASSES], f32, tag="ex", name="ex")
                nc.scalar.activation(
                    out=ex[:], in_=ps[:],
                    func=mybir.ActivationFunctionType.Exp,
                    bias=negmax[:, 0:1],
                )
                ssum = spool.tile([P, 1], f32, tag="ssum", name="ssum")
                nc.vector.tensor_reduce(
                    out=ssum[:], in_=ex[:], axis=mybir.AxisListType.X,
                    op=mybir.AluOpType.add,
                )
                rinv = spool.tile([P, 1], f32, tag="rinv", name="rinv")
                nc.vector.reciprocal(out=rinv[:], in_=ssum[:])
                prob = spool.tile([P, N_CLASSES], f32, tag="prob", name="prob")
                nc.vector.tensor_scalar_mul(prob[:], ex[:], rinv[:, 0:1])
                nc.sync.dma_start(out=out[m * P:(m + 1) * P, :], in_=prob[:])

    return nc


def build_inputs(x, edge_index, W1, b1, W2, b2, Wout, bout):
    """Host-side graph preprocessing + per-core shard construction."""
    x = np.asarray(x)
    ei = np.asarray(edge_index)
    n = N_NODES
    src = np.concatenate([ei[0], np.arange(n, dtype=np.int64)])
    dst = np.concatenate([ei[1], np.arange(n, dtype=np.int64)])
    deg = np.bincount(dst, minlength=n).astype(np.float32)
    dinv = 1.0 / np.sqrt(deg)
    normv = (dinv[src] * dinv[dst]).astype(np.float32)

    # dense Ahat^T, padded:  AhatT[src, dst] = norm  (duplicate edges sum)
    AhatT = np.zeros((NP, NP), dtype=np.float32)
    np.add.at(AhatT, (src, dst), normv)
    # DoubleRow pair-interleave: ATdr[jp, p, q, :] = AhatT[jp*256+q*128+p, :]
    ATdr = np.ascontiguousarray(
        AhatT.reshape(NJP, 2, P, NP).transpose(0, 2, 1, 3)
    ).astype(FP8)

    xp = np.zeros((NP, F_IN), dtype=np.float32)
    xp[:n] = x
    # xTt[j, p, c*128+m] = x[j*128+m, c*128+p]
    xTt = np.ascontiguousarray(
        xp.reshape(NJ, P, NF, P).transpose(0, 3, 2, 1).reshape(NJ, P, F_IN)
    ).astype(BF16)
    W1b = np.asarray(W1).astype(BF16)
    W2b = np.asarray(W2).astype(BF16)
    Woutb = np.asarray(Wout).astype(BF16)
    boutb = np.asarray(bout).reshape(1, N_CLASSES).astype(BF16)
    # biases as per-partition columns: bcols[:, l*NF + f] = b_l[f*128:(f+1)*128]
    bcols = np.stack(
        [np.asarray(b1).reshape(NF, P), np.asarray(b2).reshape(NF, P)], 0
    ).reshape(2 * NF, P).T.astype(np.float32)
    bcols = np.ascontiguousarray(bcols)

    in_maps = []
    for k in range(N_CORES):
        sl = slice(k * R, (k + 1) * R)
        in_maps.append({
            "xTt": xTt,
            "ATdr": np.ascontiguousarray(ATdr[:, :, :, sl]),
            "W1": W1b,
            "W2": W2b,
            "Wout": Woutb,
            "bcols": bcols,
            "bout": boutb,
        })
    return in_maps


_CACHED = {}


def _get_program():
    if "nc" not in _CACHED:
        nc = bass.Bass(num_devices=N_CORES)
        build_gcn(nc)
        split_drain_waits(nc)
        _CACHED["nc"] = nc
    return _CACHED["nc"]


def kernel(x, edge_index, W1, b1, W2, b2, Wout, bout, trace=False):
    install_ntff_hook()
    nc = _get_program()
    in_maps = build_inputs(x, edge_index, W1, b1, W2, b2, Wout, bout)
    res = run_bass_kernel_spmd(
        nc, in_maps, core_ids=list(range(N_CORES)), trace=trace
    )
    out = np.concatenate([res.results[k]["out"] for k in range(N_CORES)], 0)
    kernel.last_exec_time_ns = res.exec_time_ns
    kernel.last_results = res
    return out[:N_NODES].astype(np.float32)


kernel.last_exec_time_ns = None
kernel.last_results = None


# revision 6
# speedup vs baseline: 1.2452x; 1.2452x over previous
"""Trainium2 8-core GCN kernel (2-layer GCNConv + linear head + softmax).

Strategy (node/row partitioning, dense normalized adjacency):
  - Host: build Ahat = D^-1/2 (A+I) D^-1/2 as a dense fp8-e4m3 matrix, padded
    from 10000 to 10240 nodes; core k owns node rows [k*1280, (k+1)*1280).
  - Device, per core k (all matmuls fp8-e4m3 DoubleRow, fp32 accumulate):
      t1     = x @ W1 for ALL nodes (replicated GEMM; cheaper than the
               all-gather + reload stall it replaces)
      h1T_k  = relu(t1^T Ahat^T[:,k] + b1)    (transposed SpMM -> [512,1280])
      t2_k   = (h1T_k)^T @ W2                 (h1T is directly the lhsT)
      t2     = AllGather(t2_k) in 3 chunks, each launched as soon as the
               SpMM1 column pass feeding it is evacuated; SpMM2 consumes
               j-pairs in chunk-arrival order (t_pair double-buffered
               across layers so loads never wait on layer-1 readers)
      h2T_k  = relu(t2^T Ahat^T[:,k] + b2)
      out_k  = softmax(h2T_k^T @ Wout + bout) ([1280, 16] f32)
  - Host: concatenate core outputs, trim padding to [10000, 16].

The transposed SpMM (z^T = t^T A^T instead of z = A t) makes each layer's
activation land in [feature, node] layout, which is exactly the lhsT the
following GEMM needs -- no on-device transposes anywhere.  All matmuls use
perf_mode=DoubleRow (256 contraction rows per matmul): lhsT/rhs are
[128, 2, free] pair tiles, element [p, q] = contraction row q*128+p.
The SpMM's two column passes pair a 512-wide with a 256-wide chunk so the
stationary-weight load (one per (jp, f), ~214 ns) always hides under the
matmul stream.
"""

import contextlib
import ctypes
import sys
import types

import ml_dtypes
import numpy as np

import concourse.bass as bass
import concourse.mybir as mybir
import concourse.tile as tile
from concourse.bass_utils import run_bass_kernel_spmd

BF16 = ml_dtypes.bfloat16
FP8 = ml_dtypes.float8_e4m3

N_CORES = 8
N_NODES = 10000
F_IN = 512
F_HID = 512
N_CLASSES = 16
NP = 10240            # padded node count (80 * 128)
R = NP // N_CORES     # 1280 rows per core
P = 128
NJ = NP // P          # 80 contraction chunks
NJP = NJ // 2         # 40 DoubleRow contraction pairs
NM = R // P           # 10 row tiles per core
NF = F_HID // P       # 4 feature tiles
NFP = NF // 2         # 2 feature pairs
# SpMM node-column passes; each pass lists (offset, width) chunks that share
# one accumulation sweep (chunks in a pass = concurrent PSUM banks).
# Pass A pairs the 512- and 256-wide chunks so LDWEIGHTS amortizes 2x.
PASS_A = [(0, 512), (1024, 256)]
PASS_B = [(512, 512)]
# all-gather chunks (rows of ag_in): c0 = cols 0:512 (m0..3), c1 = 512:1024
# (m4..7), c2 = 1024:1280 (m8,9).  Pass A finishes c0+c2, pass B finishes c1.
AG_ROWS = [512, 512, 256]
AG_OFF = [0, 512, 1024]
AG_MTILES = [range(0, 4), range(4, 8), range(8, 10)]
AG_LAUNCH_ORDER = [0, 2, 1]

_NTFF_HOOK_INSTALLED = False


def install_ntff_hook():
    """bass_utils' trace=True path wants antenv.axon_hooks; this container
    doesn't ship it, so provide the same ctypes hook trn_boot would."""
    global _NTFF_HOOK_INSTALLED
    if _NTFF_HOOK_INSTALLED:
        return
    _NTFF_HOOK_INSTALLED = True
    try:
        lib = ctypes.CDLL("/opt/axon/libaxon_pjrt.so")
        if not hasattr(lib, "axon_start_nrt_profile"):
            return
    except OSError:
        return
    lib.axon_start_nrt_profile.argtypes = [
        ctypes.POINTER(ctypes.c_int64),
        ctypes.c_size_t,
    ]
    lib.axon_start_nrt_profile.restype = ctypes.c_int64
    lib.axon_stop_nrt_profile.argtypes = [ctypes.c_char_p]
    lib.axon_stop_nrt_profile.restype = ctypes.c_int64

    @contextlib.contextmanager
    def _hook(output_dir, device_ids):
        import jax

        jax.devices()
        if device_ids:
            ids = (ctypes.c_int64 * len(device_ids))(*device_ids)
            rc = lib.axon_start_nrt_profile(ids, len(device_ids))
        else:
            rc = lib.axon_start_nrt_profile(None, 0)
        if rc != 0:
            raise RuntimeError(f"axon_start_nrt_profile rc={rc}")
        try:
            yield
        finally:
            n = lib.axon_stop_nrt_profile(str(output_dir).encode())
            print(f"ntff profile: {n} file(s) -> {output_dir}", file=sys.stderr)

    import antenv

    mod = types.ModuleType("antenv.axon_hooks")
    mod.get_axon_ntff_profile_hook = lambda: _hook
    mod.set_axon_ntff_profile_hook = lambda h: None
    sys.modules["antenv.axon_hooks"] = mod
    antenv.axon_hooks = mod


def split_drain_waits(nc):
    """This walrus build allows only ONE sync-wait per lowered instruction
    (CTRL and pseudo-DMA structs assert on more).  Tile's wait-assignment can
    attach several; keep the last wait on the instruction and move the rest
    onto preceding single-wait NoOps on the same engine stream (waits are
    monotonic >= conditions, so enforcing them earlier in program order on
    the same engine is equivalent)."""
    for f in nc.m.functions:
        for bb in f.blocks:
            insts = bb.instructions
            i = 0
            while i < len(insts):
                inst = insts[i]
                si = getattr(inst, "sync_info", None)
                if si is not None and si.on_wait and len(si.on_wait) > 1:
                    waits = list(si.on_wait)
                    si.on_wait = [waits[-1]]
                    for j, w in enumerate(waits[:-1]):
                        pre = mybir.InstNoOp(
                            name=f"{inst.name}-presync-{j}",
                            engine=inst.engine,
                            ins=[],
                            outs=[],
                            sync_info=mybir.SyncInfo(on_wait=[w], on_update=[]),
                        )
                        insts.insert(i + j, pre)
                        nc.register_instruction(pre, overwrite=True)
                    i += len(waits) - 1
                i += 1


def build_gcn(nc):
    """Emit the SPMD GCN program (identical on every core; per-core data)."""
    f32 = mybir.dt.float32
    bf16 = mybir.dt.bfloat16
    fp8 = mybir.dt.float8e4
    rg = [list(range(N_CORES))]

    # I/O (per-core shards; same names on every core)
    # xTt8[j, p, cq, m] = x[j*128+m, cq*128+p]  (fp8; cq pairs for DoubleRow)
    xTt8 = nc.declare_dram_parameter("xTt8", [NJ, P, NF, P], fp8, isOutput=False)
    # ATdr[jp, p, q, m] = AhatT[jp*256 + q*128 + p, k*R + m]  (fp8 pairs)
    ATdr = nc.declare_dram_parameter("ATdr", [NJP, P, 2, R], fp8, isOutput=False)
    # W pair layouts: W*p8[t, p, q, n] = W[(2t+q)*128 + p, n]
    W1p = nc.declare_dram_parameter("W1p", [NFP, P, 2, F_HID], fp8, isOutput=False)
    W2p = nc.declare_dram_parameter("W2p", [NFP, P, 2, F_HID], fp8, isOutput=False)
    Woutp = nc.declare_dram_parameter("Woutp", [NFP, P, 2, N_CLASSES], fp8, isOutput=False)
    bcols = nc.declare_dram_parameter("bcols", [P, 2 * NF], f32, isOutput=False)
    bout = nc.declare_dram_parameter("bout", [1, N_CLASSES], bf16, isOutput=False)
    out = nc.declare_dram_parameter("out", [R, N_CLASSES], f32, isOutput=True)

    # layer-2 collective bounce buffers (internal DRAM), 3 chunks, fp8
    ag_in = nc.dram_tensor("ag_in", [R, F_HID], fp8)
    ag_out = [
        nc.dram_tensor(
            f"ag_out{c}", [N_CORES * AG_ROWS[c], F_HID], fp8, addr_space="Shared"
        )
        for c in range(3)
    ]

    with tile.TileContext(nc) as tc:
        with (
            tc.tile_pool(name="const", bufs=1) as cpool,
            tc.tile_pool(name="tfull", bufs=1) as tpool,
            tc.tile_pool(name="hT", bufs=1) as hpool,
            tc.tile_pool(name="work", bufs=4) as wpool,
            tc.tile_pool(name="evac", bufs=4) as epool,
            tc.tile_pool(name="sm", bufs=4) as spool,
            tc.tile_pool(name="psum", bufs=1, space="PSUM") as ppool,
        ):
            # ---- GEMM1 constants (needed immediately) ----
            W1_sb = [cpool.tile([P, 2, F_HID], fp8, tag=f"W1{t}", name=f"W1{t}") for t in range(NFP)]
            for t in range(NFP):
                nc.sync.dma_start(out=W1_sb[t][:], in_=W1p[t, :, :, :])

            # persistent activation tiles: j-PAIR tiles for DoubleRow,
            # double-buffered across layers (set 0 = t1, set 1 = t2)
            t_pair = [
                [
                    tpool.tile([P, 2, F_HID], fp8, tag=f"tp{s}_{jp}", name=f"tp{s}_{jp}")
                    for jp in range(NJP)
                ]
                for s in range(2)
            ]
            # hT as fp8 feature-pair tiles: hp[layer][t][p, q, m],
            # f-tile index ft = 2t+q  (directly the next GEMM's lhsT pairs)
            hp = [
                [hpool.tile([P, 2, R], fp8, tag=f"h{la}p{t}", name=f"h{la}p{t}") for t in range(NFP)]
                for la in range(2)
            ]

            # ---- layer 1: replicated GEMM1 (fp8 DoubleRow) ----
            for j in range(NJ):
                xt = wpool.tile([P, NF, P], fp8, tag="xtt", name="xtt")
                nc.sync.dma_start(out=xt[:], in_=xTt8[j, :, :, :])
                ps = ppool.tile([P, F_HID], f32, tag=f"sp{j % 4}", name=f"g1ps{j % 4}")
                for t in range(NFP):
                    nc.tensor.matmul(
                        out=ps[:],
                        lhsT=xt[:, 2 * t:2 * t + 2, :],
                        rhs=W1_sb[t][:, :, :],
                        start=(t == 0),
                        stop=(t == NFP - 1),
                        perf_mode=mybir.MatmulPerfMode.DoubleRow,
                    )
                nc.vector.tensor_copy(out=t_pair[0][j // 2][:, j % 2, :], in_=ps[:])

            # ---- deferred constants (needed after GEMM1 starts) ----
            W2_sb = [cpool.tile([P, 2, F_HID], fp8, tag=f"W2{t}", name=f"W2{t}") for t in range(NFP)]
            for t in range(NFP):
                nc.sync.dma_start(out=W2_sb[t][:], in_=W2p[t, :, :, :])
            Wout_sb = [cpool.tile([P, 2, N_CLASSES], fp8, tag=f"Wo{t}", name=f"Wo{t}") for t in range(NFP)]
            for t in range(NFP):
                nc.sync.dma_start(out=Wout_sb[t][:], in_=Woutp[t, :, :, :])
            bcols_sb = cpool.tile([P, 2 * NF], f32, tag="bcols", name="bcols")
            nc.sync.dma_start(out=bcols_sb[:], in_=bcols[:, :])
            bout_sb = cpool.tile([1, N_CLASSES], bf16, tag="bout", name="bout")
            nc.sync.dma_start(out=bout_sb[:], in_=bout[:, :])
            ones_sb = cpool.tile([1, P], bf16, tag="ones", name="ones")
            nc.vector.memset(ones_sb[:], 1.0)

            def spmm_pass(layer, chunks, jp_order, names):
                """One fp8 DoubleRow accumulation sweep over the given column
                chunks of hT[layer] = relu(t^T A^T + b)."""
                pstiles = {}
                bank = 0
                for (off, width) in chunks:
                    for f in range(NF):
                        pstiles[(off, f)] = ppool.tile(
                            [P, width], f32, tag=f"sp{bank}", name=f"{names}_{bank}"
                        )
                        bank += 1
                src = t_pair[layer]
                for idx, jp in enumerate(jp_order):
                    ats = {}
                    for (off, width) in chunks:
                        at = wpool.tile([P, 2, width], fp8, tag=f"at{off}", name=f"at{off}")
                        nc.scalar.dma_start(
                            out=at[:], in_=ATdr[jp, :, :, off:off + width]
                        )
                        ats[off] = at
                    for f in range(NF):
                        for (off, width) in chunks:
                            nc.tensor.matmul(
                                out=pstiles[(off, f)][:],
                                lhsT=src[jp][:, :, f * P:(f + 1) * P],
                                rhs=ats[off][:, :, :],
                                start=(idx == 0),
                                stop=(idx == NJP - 1),
                                perf_mode=mybir.MatmulPerfMode.DoubleRow,
                            )
                # evacuate: relu(psum + b) -> fp8 pair tiles; f-tile ft=2t+q
                for (off, width) in chunks:
                    for f in range(NF):
                        nc.vector.tensor_scalar(
                            out=hp[layer][f // 2][:, f % 2, off:off + width],
                            in0=pstiles[(off, f)][:],
                            scalar1=bcols_sb[:, layer * NF + f:layer * NF + f + 1],
                            scalar2=0.0,
                            op0=mybir.AluOpType.add,
                            op1=mybir.AluOpType.max,
                        )

            def gemm2_tiles(ms):
                """t2_k rows for m-tiles `ms` staged into ag_in (as fp8)."""
                for m in ms:
                    ps = ppool.tile([P, F_HID], f32, tag=f"sp{4 + m % 4}", name=f"g2ps{m % 4}")
                    for t in range(NFP):
                        nc.tensor.matmul(
                            out=ps[:],
                            lhsT=hp[0][t][:, :, m * P:(m + 1) * P],
                            rhs=W2_sb[t][:, :, :],
                            start=(t == 0),
                            stop=(t == NFP - 1),
                            perf_mode=mybir.MatmulPerfMode.DoubleRow,
                        )
                    ev = epool.tile([P, F_HID], fp8, tag="g2ev", name="g2ev")
                    nc.vector.tensor_copy(out=ev[:], in_=ps[:])
                    nc.sync.dma_start(out=ag_in[m * P:(m + 1) * P, :], in_=ev[:])

            def ag_chunk(c):
                nc.gpsimd.collective_compute(
                    "AllGather",
                    mybir.AluOpType.bypass,
                    replica_groups=rg,
                    ins=[ag_in[AG_OFF[c]:AG_OFF[c] + AG_ROWS[c], :].opt()],
                    outs=[ag_out[c][:, :].opt()],
                )

            def load_t2_chunk(c):
                # ag_out[c] rows r*AG_ROWS[c]+i*128 -> t_pair[1] j = r*10+base+i
                ntiles = AG_ROWS[c] // P
                for r in range(N_CORES):
                    for i in range(ntiles):
                        j = r * NM + AG_OFF[c] // P + i
                        row = r * AG_ROWS[c] + i * P
                        nc.sync.dma_start(
                            out=t_pair[1][j // 2][:, j % 2, :],
                            in_=ag_out[c][row:row + P, :],
                        )

            # SpMM2 consumes j-pairs in chunk-arrival order (c0, c2, c1)
            order2 = []
            for c in AG_LAUNCH_ORDER:
                for r in range(N_CORES):
                    base = r * NM + AG_OFF[c] // P
                    for jp in range(base // 2, (base + AG_ROWS[c] // P) // 2):
                        order2.append(jp)
            assert sorted(order2) == list(range(NJP))

            natural = list(range(NJP))

            # ---- layer 1 SpMM passes, GEMM2 + AG chunks interleaved ----
            spmm_pass(0, PASS_A, natural, "s1pA")
            gemm2_tiles(AG_MTILES[0])
            ag_chunk(0)
            gemm2_tiles(AG_MTILES[2])
            ag_chunk(2)
            load_t2_chunk(0)
            spmm_pass(0, PASS_B, natural, "s1pB")
            gemm2_tiles(AG_MTILES[1])
            ag_chunk(1)
            load_t2_chunk(2)
            load_t2_chunk(1)

            # ---- layer 2 SpMM ----
            spmm_pass(1, PASS_A, order2, "s2pA")
            spmm_pass(1, PASS_B, order2, "s2pB")

            # ---- output head: logits + softmax ----
            for m in range(NM):
                ps = ppool.tile([P, N_CLASSES], f32, tag=f"sp{4 + m % 4}", name=f"hps{m % 4}")
                for t in range(NFP):
                    nc.tensor.matmul(
                        out=ps[:],
                        lhsT=hp[1][t][:, :, m * P:(m + 1) * P],
                        rhs=Wout_sb[t][:, :, :],
                        start=(t == 0),
                        stop=False,
                        perf_mode=mybir.MatmulPerfMode.DoubleRow,
                    )
                nc.tensor.matmul(
                    out=ps[:],
                    lhsT=ones_sb[:, 0:P],
                    rhs=bout_sb[:],
                    start=False,
                    stop=True,
                )
                negmax = spool.tile([P, 1], f32, tag="negmax", name="negmax")
                nc.vector.tensor_reduce(
                    out=negmax[:], in_=ps[:], axis=mybir.AxisListType.X,
                    op=mybir.AluOpType.max, negate=True,
                )
                ex = spool.tile([P, N_CLASSES], f32, tag="ex", name="ex")
                nc.scalar.activation(
                    out=ex[:], in_=ps[:],
                    func=mybir.ActivationFunctionType.Exp,
                    bias=negmax[:, 0:1],
                )
                ssum = spool.tile([P, 1], f32, tag="ssum", name="ssum")
                nc.vector.tensor_reduce(
                    out=ssum[:], in_=ex[:], axis=mybir.AxisListType.X,
                    op=mybir.AluOpType.add,
                )
                rinv = spool.tile([P, 1], f32, tag="rinv", name="rinv")
                nc.vector.reciprocal(out=rinv[:], in_=ssum[:])
                prob = spool.tile([P, N_CLASSES], f32, tag="prob", name="prob")
                nc.vector.tensor_scalar_mul(prob[:], ex[:], rinv[:, 0:1])
                nc.sync.dma_start(out=out[m * P:(m + 1) * P, :], in_=prob[:])

    return nc


def build_inputs(x, edge_index, W1, b1, W2, b2, Wout, bout):
    """Host-side graph preprocessing + per-core shard construction."""
    x = np.asarray(x)
    ei = np.asarray(edge_index)
    n = N_NODES
    src = np.concatenate([ei[0], np.arange(n, dtype=np.int64)])
    dst = np.concatenate([ei[1], np.arange(n, dtype=np.int64)])
    deg = np.bincount(dst, minlength=n).astype(np.float32)
    dinv = 1.0 / np.sqrt(deg)
    normv = (dinv[src] * dinv[dst]).astype(np.float32)

    # dense Ahat^T, padded:  AhatT[src, dst] = norm  (duplicate edges sum)
    AhatT = np.zeros((NP, NP), dtype=np.float32)
    np.add.at(AhatT, (src, dst), normv)
    # DoubleRow pair-interleave: ATdr[jp, p, q, :] = AhatT[jp*256+q*128+p, :]
    ATdr = np.ascontiguousarray(
        AhatT.reshape(NJP, 2, P, NP).transpose(0, 2, 1, 3)
    ).astype(FP8)

    xp = np.zeros((NP, F_IN), dtype=np.float32)
    xp[:n] = x
    # xTt8[j, p, cq, m] = x[j*128+m, cq*128+p]
    xTt8 = np.ascontiguousarray(
        xp.reshape(NJ, P, NF, P).transpose(0, 3, 2, 1)
    ).astype(FP8)

    def wpairs(W):
        W = np.asarray(W, np.float32)
        # [t, p, q, n] = W[(2t+q)*128+p, n]
        return np.ascontiguousarray(
            W.reshape(NFP, 2, P, W.shape[1]).transpose(0, 2, 1, 3)
        ).astype(FP8)

    W1b = wpairs(W1)
    W2b = wpairs(W2)
    Woutb = wpairs(Wout)
    boutb = np.asarray(bout).reshape(1, N_CLASSES).astype(BF16)
    # biases as per-partition columns: bcols[:, l*NF + f] = b_l[f*128:(f+1)*128]
    bcols = np.stack(
        [np.asarray(b1).reshape(NF, P), np.asarray(b2).reshape(NF, P)], 0
    ).reshape(2 * NF, P).T.astype(np.float32)
    bcols = np.ascontiguousarray(bcols)

    in_maps = []
    for k in range(N_CORES):
        sl = slice(k * R, (k + 1) * R)
        in_maps.append({
            "xTt8": xTt8,
            "ATdr": np.ascontiguousarray(ATdr[:, :, :, sl]),
            "W1p": W1b,
            "W2p": W2b,
            "Woutp": Woutb,
            "bcols": bcols,
            "bout": boutb,
        })
    return in_maps


_CACHED = {}


def _get_program():
    if "nc" not in _CACHED:
        nc = bass.Bass(num_devices=N_CORES)
        build_gcn(nc)
        split_drain_waits(nc)
        _CACHED["nc"] = nc
    return _CACHED["nc"]


def kernel(x, edge_index, W1, b1, W2, b2, Wout, bout, trace=False):
    install_ntff_hook()
    nc = _get_program()
    in_maps = build_inputs(x, edge_index, W1, b1, W2, b2, Wout, bout)
    res = run_bass_kernel_spmd(
        nc, in_maps, core_ids=list(range(N_CORES)), trace=trace
    )
    out = np.concatenate([res.results[k]["out"] for k in range(N_CORES)], 0)
    kernel.last_exec_time_ns = res.exec_time_ns
    kernel.last_results = res
    return out[:N_NODES].astype(np.float32)


kernel.last_exec_time_ns = None
kernel.last_results = None
